# revision 16
# baseline (speedup 1.0000x reference)
"""Trainium2 Bass kernel for BasicMambaBlock (B=2, L=2048, d_model=1024).

Sharding: 8 cores = 2 batch groups x 4 TP shards.
Per core: d_inner shard = 512 channels, FF shard = 1024 a-cols + 1024 g-cols.
Feature-major (transposed) activation layout for the matmul chain; Mamba
recurrence via tensor_tensor_scan (channels on partitions, time on free dim).
Two in-group AllReduces (dbc partials (96,L) and out_proj partials (L,1024)).
Final FF partial sums + residual combined on host.
"""
import sys

sys.path.insert(0, "/opt/trn_rl_repo")

import numpy as np
import ml_dtypes
from contextlib import ExitStack

import concourse.bass as bass
import concourse.tile as tile
from concourse import bacc, mybir
from concourse.bass_utils import run_bass_kernel_spmd

FP32 = mybir.dt.float32
BF16 = mybir.dt.bfloat16
ALU = mybir.AluOpType
ACTF = mybir.ActivationFunctionType
NPBF16 = ml_dtypes.bfloat16

DM = 1024          # d_model
DI = 2048          # d_inner (global)
DIS = DI // 4      # 512 per-core d_inner shard
NST = 16           # d_state
RNK = 64           # dt_rank
DC = 4             # conv width
FFI = 4096         # ff inner (global)
FFS = FFI // 4     # 1024 per-core ff shard
EPS = 1e-5
L_FULL = 2048
B_FULL = 2

# When True, emit compositions of sim-supported ACT funcs instead of
# Silu/Softplus/Gelu (the CoreSim interpreter lacks those LUTs).
SIM_SAFE = False


def _act_silu(nc, scr, out, in_, bias=None):
    """out = silu(in_ + bias)."""
    if not SIM_SAFE:
        nc.scalar.activation(out, in_, ACTF.Silu,
                             bias=(bias if bias is not None else 0.0))
        return
    shape = [in_.shape[0], in_.free_size()]
    v = scr.tile(shape, FP32, tag="silu_v")
    nc.scalar.activation(v[:], in_, ACTF.Identity,
                         bias=(bias if bias is not None else 0.0))
    s = scr.tile(shape, FP32, tag="silu_s")
    nc.scalar.activation(s[:], v[:], ACTF.Sigmoid)
    nc.vector.tensor_mul(out, v[:], s[:])


def _act_softplus(nc, scr, out, in_, bias):
    """out = softplus(in_ + bias) = ln(1 + exp(in_ + bias)).

    Composed from Exp+Ln (same ACT table set as the scan's Exp) because
    this walrus build has no Softplus LUT set."""
    shape = [in_.shape[0], in_.free_size()]
    e = scr.tile(shape, FP32, tag="sp_e")
    nc.scalar.activation(e[:], in_, ACTF.Exp, bias=bias)
    nc.vector.tensor_scalar_add(e[:], e[:], 1.0)
    nc.scalar.activation(out, e[:], ACTF.Ln)


def _act_gelu(nc, scr, out, in_, bias):
    """out = gelu_tanh(in_ + bias)."""
    if not SIM_SAFE:
        nc.scalar.activation(out, in_, ACTF.Gelu_apprx_tanh, bias=bias)
        return
    shape = [in_.shape[0], in_.free_size()]
    v = scr.tile(shape, FP32, tag="ge_v")
    nc.scalar.activation(v[:], in_, ACTF.Identity, bias=bias)
    v2 = scr.tile(shape, FP32, tag="ge_v2")
    nc.scalar.activation(v2[:], v[:], ACTF.Square)
    v3 = scr.tile(shape, FP32, tag="ge_v3")
    nc.vector.tensor_mul(v3[:], v2[:], v[:])
    u = scr.tile(shape, FP32, tag="ge_u")
    nc.vector.scalar_tensor_tensor(u[:], v3[:], 0.044715, v[:],
                                   ALU.mult, ALU.add)
    w = scr.tile(shape, FP32, tag="ge_w")
    nc.scalar.activation(w[:], u[:], ACTF.Tanh, scale=0.7978845608028654)
    nc.vector.tensor_scalar(w[:], w[:], 1.0, 0.5, ALU.add, ALU.mult)
    nc.vector.tensor_mul(out, v[:], w[:])


def _layer_norm_stage(nc, tc, ctx, src_tiles, n_tok_tiles, ident_sb, g_ap, b_ap,
                      hfm_pool, L, name):
    """Token-major LN on src_tiles (list of (128, DM) fp32 sbuf tiles) ->
    feature-major bf16 tiles (8 x (128, L)), with g/b applied per-partition
    after the transpose. Returns list of 8 hfm tiles."""
    stat = ctx.enter_context(tc.tile_pool(name=f"{name}_stat", bufs=4))
    scr = ctx.enter_context(tc.tile_pool(name=f"{name}_scr", bufs=2))
    nrm = ctx.enter_context(tc.tile_pool(name=f"{name}_nrm", bufs=n_tok_tiles))
    gsb = ctx.enter_context(tc.tile_pool(name=f"{name}_gb", bufs=1))

    # g/b per-feature: 8 x (128,1) tiles
    g_t, b_t = [], []
    for f in range(DM // 128):
        t = gsb.tile([128, 1], FP32, tag=f"g{f}")
        nc.sync.dma_start(out=t[:], in_=g_ap[f * 128:(f + 1) * 128, :])
        g_t.append(t)
        t = gsb.tile([128, 1], FP32, tag=f"b{f}")
        nc.sync.dma_start(out=t[:], in_=b_ap[f * 128:(f + 1) * 128, :])
        b_t.append(t)

    eps_t = gsb.tile([128, 1], FP32, tag="eps")
    nc.vector.memset(eps_t[:], EPS)

    normed = []
    for i in range(n_tok_tiles):
        xt = src_tiles[i]
        s1 = stat.tile([128, 1], FP32, tag="s1")
        nc.vector.tensor_reduce(s1[:], xt[:], mybir.AxisListType.X, ALU.add)
        sq = scr.tile([128, DM], FP32, tag="sq")
        s2 = stat.tile([128, 1], FP32, tag="s2")
        nc.scalar.activation(sq[:], xt[:], ACTF.Square, accum_out=s2[:])
        mu = stat.tile([128, 1], FP32, tag="mu")
        nc.vector.tensor_scalar_mul(mu[:], s1[:], 1.0 / DM)
        var = stat.tile([128, 1], FP32, tag="var")
        # var = s2/DM - mu^2
        musq = stat.tile([128, 1], FP32, tag="musq")
        nc.vector.tensor_mul(musq[:], mu[:], mu[:])
        nc.vector.tensor_scalar(var[:], s2[:], 1.0 / DM, None, ALU.mult)
        nc.vector.tensor_sub(var[:], var[:], musq[:])
        lv = stat.tile([128, 1], FP32, tag="lv")
        nc.scalar.activation(lv[:], var[:], ACTF.Ln, bias=eps_t[:])
        rstd = stat.tile([128, 1], FP32, tag="rstd")
        nc.scalar.activation(rstd[:], lv[:], ACTF.Exp, scale=-0.5)
        nt = nrm.tile([128, DM], BF16, tag="normed")
        nc.vector.tensor_scalar(nt[:], xt[:], mu[:], rstd[:],
                                ALU.subtract, ALU.mult)
        normed.append(nt)

    # transpose to feature-major; fuse g/b at PSUM evacuation
    psT = ctx.enter_context(tc.tile_pool(name=f"{name}_psT", bufs=2,
                                         space="PSUM"))
    hfm = []
    for f in range(DM // 128):
        pt = psT.tile([128, L], BF16, tag="psT")
        for i in range(n_tok_tiles):
            nc.tensor.transpose(pt[:, i * 128:(i + 1) * 128],
                                normed[i][:, f * 128:(f + 1) * 128],
                                ident_sb[:])
        ht = hfm_pool.tile([128, L], BF16, tag="hfm")
        nc.any.tensor_scalar(ht[:], pt[:], g_t[f][:], b_t[f][:],
                             ALU.mult, ALU.add)
        hfm.append(ht)
    return hfm


def build_nc(L=L_FULL):
    n_tok = L // 128
    CH = min(512, L)
    n_ch = L // CH  # token chunks for matmul moving dim

    nc = bacc.Bacc("TRN2", target_bir_lowering=False, debug=False,
                   num_devices=8)

    # ---- dram params ----
    def din(name, shape, dt=FP32):
        return nc.dram_tensor(name, shape, dt, kind="ExternalInput").ap()

    x_d = din("x", [L, DM])
    ln1_g = din("ln1_g", [DM, 1]); ln1_b = din("ln1_b", [DM, 1])
    ln2_g = din("ln2_g", [DM, 1]); ln2_b = din("ln2_b", [DM, 1])
    w_in_d = din("w_in", [DM, 2 * DIS], BF16)      # [xc cols | z cols]
    conv_w_d = din("conv_w", [DIS, DC])
    conv_b_d = din("conv_b", [DIS, 1])
    a_neg_d = din("a_neg", [DIS, NST])             # A = -exp(a_log) shard
    w_x_d = din("w_x", [DIS, RNK + 2 * NST], BF16)
    w_dt_d = din("w_dt", [RNK, DIS], BF16)
    b_dt_d = din("b_dt", [DIS, 1])
    d_skip_d = din("d_skip", [DIS, 1])
    w_out_d = din("w_out", [DIS, DM], BF16)
    w_ff1_d = din("w_ff1", [DM, 2 * FFS], BF16)    # [a cols | g cols]
    b_ff1_d = din("b_ff1", [2 * FFS, 1])
    w_ff2_d = din("w_ff2", [FFS, DM], BF16)
    ident_d = din("ident", [128, 128], BF16)

    x2_out = nc.dram_tensor("x2_out", [L, DM], FP32,
                            kind="ExternalOutput").ap()
    ffp_out = nc.dram_tensor("ffp_out", [L, DM], FP32,
                             kind="ExternalOutput").ap()

    with tile.TileContext(nc) as tc, ExitStack() as octx:
        dram = octx.enter_context(tc.tile_pool(name="dram", bufs=1,
                                               space="DRAM"))
        mm = octx.enter_context(tc.tile_pool(name="mm", bufs=4, space="PSUM"))
        const = octx.enter_context(tc.tile_pool(name="const", bufs=1))

        ident_sb = const.tile([128, 128], BF16, tag="ident")
        nc.sync.dma_start(out=ident_sb[:], in_=ident_d[:, :])

        # dram intermediates
        dbc_part = dram.tile([RNK + 2 * NST, L], FP32, tag="dbc_part")
        dbc_ar = dram.tile([RNK + 2 * NST, L], FP32, tag="dbc_ar")
        bbf_d = dram.tile([NST, L], BF16, tag="bbf")
        m_part = dram.tile([L, DM], FP32, tag="m_part")
        m_ar = dram.tile([L, DM], FP32, tag="m_ar")

        groups = [[0, 1, 2, 3], [4, 5, 6, 7]]

        # Long-lived pools (phases 2-4) created first so shorter-lived
        # pools can pop in LIFO order before phase 5 reuses the space.
        pMain = octx.enter_context(ExitStack())
        wts = pMain.enter_context(tc.tile_pool(name="wts", bufs=1))
        sconst = pMain.enter_context(tc.tile_pool(name="sconst", bufs=1))
        act = pMain.enter_context(tc.tile_pool(name="act", bufs=1))
        dtp = pMain.enter_context(tc.tile_pool(name="dtp", bufs=1))

        # ================= Phase 1: LN1 -> h_fm =================
        p12 = pMain.enter_context(ExitStack())
        hfm_pool = p12.enter_context(tc.tile_pool(name="hfm", bufs=8))
        with ExitStack() as p1:
            xload = p1.enter_context(tc.tile_pool(name="xload", bufs=3))
            xt_list = []
            for i in range(n_tok):
                xt = xload.tile([128, DM], FP32, tag="xt")
                nc.sync.dma_start(out=xt[:], in_=x_d[i * 128:(i + 1) * 128, :])
                xt_list.append(xt)
            # NOTE: xload bufs=3 but we keep refs; tiles with same tag share 3
            # slots -> must consume before reuse. LN consumes immediately, but
            # transposes need all normed tiles (not xt). OK.
            hfm = _layer_norm_stage(nc, tc, p1, xt_list, n_tok, ident_sb,
                                    ln1_g, ln1_b, hfm_pool, L, "ln1")

        # ================= Phase 2: in_proj, conv, dbc, dt =================
        p2 = p12
        w_in_sb = []
        for k in range(8):
            t = wts.tile([128, 2 * DIS], BF16, tag=f"w_in{k}")
            nc.sync.dma_start(out=t[:], in_=w_in_d[k * 128:(k + 1) * 128, :])
            w_in_sb.append(t)
        wx_sb = []
        for k in range(4):
            t = wts.tile([128, RNK + 2 * NST], BF16, tag=f"wx{k}")
            nc.sync.dma_start(out=t[:], in_=w_x_d[k * 128:(k + 1) * 128, :])
            wx_sb.append(t)
        wdt_sb = wts.tile([RNK, DIS], BF16, tag="wdt")
        nc.sync.dma_start(out=wdt_sb[:], in_=w_dt_d[:, :])

        cw_sb, cb_sb, a_sb, bdt_sb, dskip_sb = [], [], [], [], []
        for d in range(4):
            r = slice(d * 128, (d + 1) * 128)
            t = sconst.tile([128, DC], FP32, tag=f"cw{d}")
            nc.sync.dma_start(out=t[:], in_=conv_w_d[r, :]); cw_sb.append(t)
            t = sconst.tile([128, 1], FP32, tag=f"cb{d}")
            nc.sync.dma_start(out=t[:], in_=conv_b_d[r, :]); cb_sb.append(t)
            t = sconst.tile([128, NST], FP32, tag=f"a{d}")
            nc.sync.dma_start(out=t[:], in_=a_neg_d[r, :]); a_sb.append(t)
            t = sconst.tile([128, 1], FP32, tag=f"bdt{d}")
            nc.sync.dma_start(out=t[:], in_=b_dt_d[r, :]); bdt_sb.append(t)
            t = sconst.tile([128, 1], FP32, tag=f"dsk{d}")
            nc.sync.dma_start(out=t[:], in_=d_skip_d[r, :]); dskip_sb.append(t)

        xc_pad, z_sb = [], []
        for d in range(4):
            t = act.tile([128, L + 3], BF16, tag=f"xcp{d}")
            nc.vector.memset(t[:, 0:3], 0.0)
            xc_pad.append(t)
            t = act.tile([128, L], BF16, tag=f"z{d}")
            z_sb.append(t)

        # in_proj: out feature tile f (0..3 -> xc, 4..7 -> z)
        for f in range(8):
            for c in range(n_ch):
                ps = mm.tile([128, CH], FP32, tag="mm")
                for k in range(8):
                    nc.tensor.matmul(
                        ps[:], w_in_sb[k][:, f * 128:(f + 1) * 128],
                        hfm[k][:, c * CH:(c + 1) * CH],
                        start=(k == 0), stop=(k == 7))
                if f < 4:
                    dst = xc_pad[f][:, 3 + c * CH: 3 + (c + 1) * CH]
                else:
                    dst = z_sb[f - 4][:, c * CH:(c + 1) * CH]
                nc.any.tensor_copy(dst, ps[:])

        # conv + silu (writes silu'd xc back into xc_pad[:, 3:3+L])
        cacc = p2.enter_context(tc.tile_pool(name="cacc", bufs=2))
        for d in range(4):
            acc = cacc.tile([128, L], BF16, tag="cacc")
            nc.vector.tensor_scalar_mul(acc[:], xc_pad[d][:, 0:L],
                                        cw_sb[d][:, 0:1])
            for j in range(1, DC):
                nc.vector.scalar_tensor_tensor(
                    acc[:], xc_pad[d][:, j:j + L], cw_sb[d][:, j:j + 1],
                    acc[:], ALU.mult, ALU.add)
            _act_silu(nc, cacc, xc_pad[d][:, 3:3 + L], acc[:],
                      bias=cb_sb[d][:])

        # dbc partial + AllReduce
        dbcp = p2.enter_context(tc.tile_pool(name="dbcp", bufs=1))
        dbc_sb = dbcp.tile([RNK + 2 * NST, L], FP32, tag="dbc")
        for c in range(n_ch):
            ps = mm.tile([RNK + 2 * NST, CH], FP32, tag="mm")
            for k in range(4):
                nc.tensor.matmul(ps[:], wx_sb[k][:],
                                 xc_pad[k][:, 3 + c * CH:3 + (c + 1) * CH],
                                 start=(k == 0), stop=(k == 3))
            nc.any.tensor_copy(dbc_sb[:, c * CH:(c + 1) * CH], ps[:])
        nc.gpsimd.dma_start(out=dbc_part[:], in_=dbc_sb[:])
        nc.gpsimd.collective_compute(
            "AllReduce", ALU.add, replica_groups=groups,
            ins=[dbc_part.opt()], outs=[dbc_ar.opt()])
        nc.sync.dma_start(out=dbc_sb[:], in_=dbc_ar[:])

        # dt_lo bf16 cast; B rows bf16 to dram for broadcast
        misc = p2.enter_context(tc.tile_pool(name="misc", bufs=1))
        dtlo = misc.tile([RNK, L], BF16, tag="dtlo")
        nc.any.tensor_copy(dtlo[:], dbc_sb[0:RNK, :])
        bbf = misc.tile([NST, L], BF16, tag="bbf")
        nc.any.tensor_copy(bbf[:], dbc_sb[RNK:RNK + NST, :])
        nc.sync.dma_start(out=bbf_d[:], in_=bbf[:])

        # dt = softplus(dt_lo @ w_dt + b_dt)  (bf16 out)
        dt_sb = []
        for d in range(4):
            t = dtp.tile([128, L], BF16, tag=f"dt{d}")
            dt_sb.append(t)
            for c in range(n_ch):
                ps = mm.tile([128, CH], FP32, tag="mm")
                nc.tensor.matmul(ps[:], wdt_sb[:, d * 128:(d + 1) * 128],
                                 dtlo[:, c * CH:(c + 1) * CH],
                                 start=True, stop=True)
                _act_softplus(nc, dtp, t[:, c * CH:(c + 1) * CH], ps[:],
                              bdt_sb[d][:])

        p12.close()

        # ================= Phase 3: scan + gate =================
        p3 = pMain.enter_context(ExitStack())
        sc = p3.enter_context(tc.tile_pool(name="scan", bufs=2))
        bcp = p3.enter_context(tc.tile_pool(name="bcast", bufs=2))
        yp = p3.enter_context(tc.tile_pool(name="y", bufs=2))
        gp = p3.enter_context(tc.tile_pool(name="gated", bufs=4))
        gated_sb = []
        for d in range(4):
            dtx = sc.tile([128, L], BF16, tag="dtx")
            nc.vector.tensor_mul(dtx[:], dt_sb[d][:], xc_pad[d][:, 3:3 + L])
            y_t = yp.tile([128, L], FP32, tag="y")
            for n in range(NST):
                bcB = bcp.tile([128, L], BF16, tag="bcB")
                nc.sync.dma_start(
                    out=bcB[:],
                    in_=bbf_d[n:n + 1, :].broadcast_to([128, L]))
                bcC = bcp.tile([128, L], FP32, tag="bcC")
                nc.sync.dma_start(
                    out=bcC[:],
                    in_=dbc_ar[RNK + NST + n:RNK + NST + n + 1,
                               :].broadcast_to([128, L]))
                dA = sc.tile([128, L], FP32, tag="dA")
                nc.scalar.activation(dA[:], dt_sb[d][:], ACTF.Exp,
                                     scale=a_sb[d][:, n:n + 1])
                dB = sc.tile([128, L], FP32, tag="dB")
                nc.vector.tensor_mul(dB[:], dtx[:], bcB[:])
                # h (in-place over dB): state = dA*state + dB
                nc.vector.tensor_tensor_scan(dB[:], dA[:], dB[:], 0.0,
                                             ALU.mult, ALU.add)
                if n == 0:
                    nc.gpsimd.tensor_mul(y_t[:], dB[:], bcC[:])
                else:
                    nc.gpsimd.tensor_mul(dA[:], dB[:], bcC[:])  # ch into dA
                    nc.vector.tensor_add(y_t[:], y_t[:], dA[:])
            # gate: gated = (y + d_skip*xc) * silu(z)
            sz = sc.tile([128, L], FP32, tag="dB")
            _act_silu(nc, sc, sz[:], z_sb[d][:])
            tmp = sc.tile([128, L], FP32, tag="dA")
            nc.vector.tensor_scalar_mul(tmp[:], xc_pad[d][:, 3:3 + L],
                                        dskip_sb[d][:])
            nc.vector.tensor_add(y_t[:], y_t[:], tmp[:])
            gt = gp.tile([128, L], BF16, tag="gated")
            nc.vector.tensor_mul(gt[:], y_t[:], sz[:])
            gated_sb.append(gt)

        # ================= Phase 4: out_proj + AR2 =================
        wout_sb = []
        for k in range(4):
            t = wts.tile([128, DM], BF16, tag=f"wout{k}")
            nc.sync.dma_start(out=t[:], in_=w_out_d[k * 128:(k + 1) * 128, :])
            wout_sb.append(t)
        mp_pool = p3.enter_context(tc.tile_pool(name="mp", bufs=2))
        for i in range(n_tok):
            mp = mp_pool.tile([128, DM], FP32, tag="mp")
            for nchk in range(2):
                ps = mm.tile([128, 512], FP32, tag="mm")
                for k in range(4):
                    nc.tensor.matmul(
                        ps[:], gated_sb[k][:, i * 128:(i + 1) * 128],
                        wout_sb[k][:, nchk * 512:(nchk + 1) * 512],
                        start=(k == 0), stop=(k == 3))
                nc.any.tensor_copy(mp[:, nchk * 512:(nchk + 1) * 512], ps[:])
            nc.sync.dma_start(out=m_part[i * 128:(i + 1) * 128, :], in_=mp[:])
        nc.gpsimd.collective_compute(
            "AllReduce", ALU.add, replica_groups=groups,
            ins=[m_part.opt()], outs=[m_ar.opt()])
        p3.close()
        pMain.close()

        # ================= Phase 5: residual + LN2 =================
        h2fm_pool = octx.enter_context(tc.tile_pool(name="h2fm", bufs=8))
        with ExitStack() as p5:
            x2p = p5.enter_context(tc.tile_pool(name="x2", bufs=4))
            ld = p5.enter_context(tc.tile_pool(name="ld5", bufs=3))
            x2_list = []
            for i in range(n_tok):
                r = slice(i * 128, (i + 1) * 128)
                xt = ld.tile([128, DM], FP32, tag="xr")
                nc.sync.dma_start(out=xt[:], in_=x_d[r, :])
                mt = ld.tile([128, DM], FP32, tag="mr")
                nc.sync.dma_start(out=mt[:], in_=m_ar[r, :])
                x2 = x2p.tile([128, DM], FP32, tag="x2")
                nc.vector.tensor_add(x2[:], xt[:], mt[:])
                nc.sync.dma_start(out=x2_out[r, :], in_=x2[:])
                x2_list.append(x2)
            h2fm = _layer_norm_stage(nc, tc, p5, x2_list, n_tok, ident_sb,
                                     ln2_g, ln2_b, h2fm_pool, L, "ln2")

        # ================= Phase 6: FF =================
        with ExitStack() as p6:
            wf = p6.enter_context(tc.tile_pool(name="wf", bufs=1))
            wff1_sb = []
            for k in range(8):
                t = wf.tile([128, 2 * FFS], BF16, tag=f"wff1{k}")
                nc.sync.dma_start(out=t[:],
                                  in_=w_ff1_d[k * 128:(k + 1) * 128, :])
                wff1_sb.append(t)
            wff2_sb = []
            for k in range(8):
                t = wf.tile([128, DM], BF16, tag=f"wff2{k}")
                nc.sync.dma_start(out=t[:],
                                  in_=w_ff2_d[k * 128:(k + 1) * 128, :])
                wff2_sb.append(t)
            bf1 = []
            for j in range(2 * FFS // 128):
                t = wf.tile([128, 1], FP32, tag=f"bf1_{j}")
                nc.sync.dma_start(out=t[:], in_=b_ff1_d[j * 128:(j + 1) * 128, :])
                bf1.append(t)

            agp = p6.enter_context(tc.tile_pool(name="ag", bufs=8))
            tmp6 = p6.enter_context(tc.tile_pool(name="tmp6", bufs=4))
            ag_sb = []
            for fa in range(8):
                agt = agp.tile([128, L], BF16, tag="ag")
                ag_sb.append(agt)
                for c in range(n_ch):
                    psA = mm.tile([128, CH], FP32, tag="mm")
                    for k in range(8):
                        nc.tensor.matmul(
                            psA[:], wff1_sb[k][:, fa * 128:(fa + 1) * 128],
                            h2fm[k][:, c * CH:(c + 1) * CH],
                            start=(k == 0), stop=(k == 7))
                    psG = mm.tile([128, CH], FP32, tag="mm")
                    for k in range(8):
                        nc.tensor.matmul(
                            psG[:],
                            wff1_sb[k][:, FFS + fa * 128:FFS + (fa + 1) * 128],
                            h2fm[k][:, c * CH:(c + 1) * CH],
                            start=(k == 0), stop=(k == 7))
                    aa = tmp6.tile([128, CH], BF16, tag="aa")
                    nc.scalar.activation(aa[:], psA[:], ACTF.Identity,
                                         bias=bf1[fa][:])
                    gg = tmp6.tile([128, CH], BF16, tag="gg")
                    _act_gelu(nc, tmp6, gg[:], psG[:],
                              bf1[FFS // 128 + fa][:])
                    nc.vector.tensor_mul(agt[:, c * CH:(c + 1) * CH],
                                         aa[:], gg[:])

            ffo = p6.enter_context(tc.tile_pool(name="ffo", bufs=2))
            for i in range(n_tok):
                fo = ffo.tile([128, DM], FP32, tag="fo")
                for nchk in range(2):
                    ps = mm.tile([128, 512], FP32, tag="mm")
                    for k in range(8):
                        nc.tensor.matmul(
                            ps[:], ag_sb[k][:, i * 128:(i + 1) * 128],
                            wff2_sb[k][:, nchk * 512:(nchk + 1) * 512],
                            start=(k == 0), stop=(k == 7))
                    nc.any.tensor_copy(fo[:, nchk * 512:(nchk + 1) * 512],
                                       ps[:])
                nc.sync.dma_start(out=ffp_out[i * 128:(i + 1) * 128, :],
                                  in_=fo[:])
    nc.compile()
    return nc


_NC_CACHE = {}


def _get_nc(L=L_FULL):
    if L not in _NC_CACHE:
        _NC_CACHE[L] = build_nc(L)
    return _NC_CACHE[L]


def make_in_maps(x, ln1_g, ln1_b, w_in, conv_w, conv_b, w_x, w_dt, b_dt,
                 a_log, d_skip, w_out, ln2_g, ln2_b, w_ff1, b_ff1, w_ff2,
                 b_ff2):
    x = np.asarray(x, np.float32)
    f32 = lambda a: np.ascontiguousarray(np.asarray(a, np.float32))
    bf = lambda a: np.ascontiguousarray(np.asarray(a, np.float32)).astype(NPBF16)
    ident = np.eye(128, dtype=np.float32).astype(NPBF16)
    a_neg = -np.exp(np.asarray(a_log, np.float32))
    in_maps = []
    for c in range(8):
        b, s = c // 4, c % 4
        ds = slice(s * DIS, (s + 1) * DIS)
        fs = slice(s * FFS, (s + 1) * FFS)
        gs = slice(FFI + s * FFS, FFI + (s + 1) * FFS)
        in_maps.append(dict(
            x=f32(x[b]),
            ln1_g=f32(ln1_g).reshape(DM, 1), ln1_b=f32(ln1_b).reshape(DM, 1),
            ln2_g=f32(ln2_g).reshape(DM, 1), ln2_b=f32(ln2_b).reshape(DM, 1),
            w_in=bf(np.concatenate(
                [w_in[:, s * DIS:(s + 1) * DIS],
                 w_in[:, DI + s * DIS:DI + (s + 1) * DIS]], axis=1)),
            conv_w=f32(conv_w[ds]), conv_b=f32(conv_b[ds]).reshape(DIS, 1),
            a_neg=f32(a_neg[ds]),
            w_x=bf(w_x[ds]), w_dt=bf(w_dt[:, ds]),
            b_dt=f32(b_dt[ds]).reshape(DIS, 1),
            d_skip=f32(d_skip[ds]).reshape(DIS, 1),
            w_out=bf(w_out[ds]),
            w_ff1=bf(np.concatenate([w_ff1[:, fs], w_ff1[:, gs]], axis=1)),
            b_ff1=f32(np.concatenate(
                [b_ff1[s * FFS:(s + 1) * FFS],
                 b_ff1[FFI + s * FFS:FFI + (s + 1) * FFS]])).reshape(2 * FFS, 1),
            w_ff2=bf(w_ff2[fs]),
            ident=ident,
        ))
    return in_maps


def combine_outputs(results, b_ff2, L=L_FULL):
    out = np.zeros((B_FULL, L, DM), np.float32)
    bff2 = np.asarray(b_ff2, np.float32)
    for b in range(B_FULL):
        acc = results[4 * b]["x2_out"].astype(np.float32).copy()
        for s in range(4):
            acc += results[4 * b + s]["ffp_out"].astype(np.float32)
        out[b] = acc + bff2[None, :]
    return out


def kernel(**inputs):
    nc = _get_nc(L_FULL)
    in_maps = make_in_maps(
        inputs["x"], inputs["ln1_g"], inputs["ln1_b"], inputs["w_in"],
        inputs["conv_w"], inputs["conv_b"], inputs["w_x"], inputs["w_dt"],
        inputs["b_dt"], inputs["a_log"], inputs["d_skip"], inputs["w_out"],
        inputs["ln2_g"], inputs["ln2_b"], inputs["w_ff1"], inputs["b_ff1"],
        inputs["w_ff2"], inputs["b_ff2"])
    res = run_bass_kernel_spmd(nc, in_maps, core_ids=list(range(8)))
    return combine_outputs(res.results, inputs["b_ff2"], L_FULL)


# revision 22
# speedup vs baseline: 26.2786x; 26.2786x over previous
"""Trainium2 Bass kernel for BasicMambaBlock (B=2, L=2048, d_model=1024).

Sharding: 8 cores = 2 batch groups x 4 TP shards.
Per core: d_inner shard = 512 channels, FF shard = 1024 a-cols + 1024 g-cols.
Feature-major (transposed) activation layout for the matmul chain; Mamba
recurrence via tensor_tensor_scan (channels on partitions, time on free dim).
Two in-group AllReduces (dbc partials (96,L) and out_proj partials (L,1024)).
Final FF partial sums + residual combined on host.
"""
import sys

sys.path.insert(0, "/opt/trn_rl_repo")

import numpy as np
import ml_dtypes
from contextlib import ExitStack

import concourse.bass as bass
import concourse.tile as tile
from concourse import bacc, mybir
from concourse.bass_utils import run_bass_kernel_spmd

FP32 = mybir.dt.float32
BF16 = mybir.dt.bfloat16
ALU = mybir.AluOpType
ACTF = mybir.ActivationFunctionType
NPBF16 = ml_dtypes.bfloat16

DM = 1024          # d_model
DI = 2048          # d_inner (global)
DIS = DI // 4      # 512 per-core d_inner shard
NST = 16           # d_state
RNK = 64           # dt_rank
DC = 4             # conv width
FFI = 4096         # ff inner (global)
FFS = FFI // 4     # 1024 per-core ff shard
EPS = 1e-5
L_FULL = 2048
B_FULL = 2

# When True, emit compositions of sim-supported ACT funcs instead of
# Silu/Softplus/Gelu (the CoreSim interpreter lacks those LUTs).
SIM_SAFE = False


def _act_silu(nc, scr, out, in_, bias=None):
    """out = silu(in_ + bias)."""
    if not SIM_SAFE:
        nc.scalar.activation(out, in_, ACTF.Silu,
                             bias=(bias if bias is not None else 0.0))
        return
    shape = [in_.shape[0], in_.free_size()]
    v = scr.tile(shape, FP32, tag="silu_v")
    nc.scalar.activation(v[:], in_, ACTF.Identity,
                         bias=(bias if bias is not None else 0.0))
    s = scr.tile(shape, FP32, tag="silu_s")
    nc.scalar.activation(s[:], v[:], ACTF.Sigmoid)
    nc.vector.tensor_mul(out, v[:], s[:])


def _act_softplus(nc, scr, out, in_, bias):
    """out = softplus(in_ + bias) = ln(1 + exp(in_ + bias)).

    Composed from Exp+Ln (same ACT table set as the scan's Exp) because
    this walrus build has no Softplus LUT set."""
    shape = [in_.shape[0], in_.free_size()]
    e = scr.tile(shape, FP32, tag="sp_e")
    nc.scalar.activation(e[:], in_, ACTF.Exp, bias=bias)
    nc.vector.tensor_scalar_add(e[:], e[:], 1.0)
    nc.scalar.activation(out, e[:], ACTF.Ln)


def _act_gelu(nc, scr, out, in_, bias):
    """out = gelu_tanh(in_ + bias)."""
    if not SIM_SAFE:
        nc.scalar.activation(out, in_, ACTF.Gelu_apprx_tanh, bias=bias)
        return
    shape = [in_.shape[0], in_.free_size()]
    v = scr.tile(shape, FP32, tag="ge_v")
    nc.scalar.activation(v[:], in_, ACTF.Identity, bias=bias)
    v2 = scr.tile(shape, FP32, tag="ge_v2")
    nc.scalar.activation(v2[:], v[:], ACTF.Square)
    v3 = scr.tile(shape, FP32, tag="ge_v3")
    nc.vector.tensor_mul(v3[:], v2[:], v[:])
    u = scr.tile(shape, FP32, tag="ge_u")
    nc.vector.scalar_tensor_tensor(u[:], v3[:], 0.044715, v[:],
                                   ALU.mult, ALU.add)
    w = scr.tile(shape, FP32, tag="ge_w")
    nc.scalar.activation(w[:], u[:], ACTF.Tanh, scale=0.7978845608028654)
    nc.vector.tensor_scalar(w[:], w[:], 1.0, 0.5, ALU.add, ALU.mult)
    nc.vector.tensor_mul(out, v[:], w[:])


def _layer_norm_stage(nc, tc, ctx, src_tiles, n_tok_tiles, ident_sb, g_ap, b_ap,
                      hfm_pool, L, name):
    """Token-major LN on src_tiles (list of (128, DM) fp32 sbuf tiles) ->
    feature-major bf16 tiles (8 x (128, L)), with g/b applied per-partition
    after the transpose. Returns list of 8 hfm tiles."""
    stat = ctx.enter_context(tc.tile_pool(name=f"{name}_stat", bufs=4))
    scr = ctx.enter_context(tc.tile_pool(name=f"{name}_scr", bufs=1))
    nrm = ctx.enter_context(tc.tile_pool(name=f"{name}_nrm", bufs=n_tok_tiles))
    gsb = ctx.enter_context(tc.tile_pool(name=f"{name}_gb", bufs=1))

    # g/b per-feature: 8 x (128,1) tiles
    g_t, b_t = [], []
    for f in range(DM // 128):
        t = gsb.tile([128, 1], FP32, tag=f"g{f}")
        nc.sync.dma_start(out=t[:], in_=g_ap[f * 128:(f + 1) * 128, :])
        g_t.append(t)
        t = gsb.tile([128, 1], FP32, tag=f"b{f}")
        nc.sync.dma_start(out=t[:], in_=b_ap[f * 128:(f + 1) * 128, :])
        b_t.append(t)

    eps_t = gsb.tile([128, 1], FP32, tag="eps")
    nc.vector.memset(eps_t[:], EPS)

    normed = []
    for i in range(n_tok_tiles):
        xt = src_tiles[i]
        s1 = stat.tile([128, 1], FP32, tag="s1")
        nc.vector.tensor_reduce(s1[:], xt[:], mybir.AxisListType.X, ALU.add)
        sq = scr.tile([128, DM], FP32, tag="sq")
        s2 = stat.tile([128, 1], FP32, tag="s2")
        nc.scalar.activation(sq[:], xt[:], ACTF.Square, accum_out=s2[:])
        mu = stat.tile([128, 1], FP32, tag="mu")
        nc.vector.tensor_scalar_mul(mu[:], s1[:], 1.0 / DM)
        var = stat.tile([128, 1], FP32, tag="var")
        # var = s2/DM - mu^2
        musq = stat.tile([128, 1], FP32, tag="musq")
        nc.vector.tensor_mul(musq[:], mu[:], mu[:])
        nc.vector.tensor_scalar(var[:], s2[:], 1.0 / DM, None, ALU.mult)
        nc.vector.tensor_sub(var[:], var[:], musq[:])
        lv = stat.tile([128, 1], FP32, tag="lv")
        nc.scalar.activation(lv[:], var[:], ACTF.Ln, bias=eps_t[:])
        rstd = stat.tile([128, 1], FP32, tag="rstd")
        nc.scalar.activation(rstd[:], lv[:], ACTF.Exp, scale=-0.5)
        nt = nrm.tile([128, DM], BF16, tag="normed")
        nc.vector.tensor_scalar(nt[:], xt[:], mu[:], rstd[:],
                                ALU.subtract, ALU.mult)
        normed.append(nt)

    # transpose to feature-major; fuse g/b at PSUM evacuation
    psT = ctx.enter_context(tc.tile_pool(name=f"{name}_psT", bufs=2,
                                         space="PSUM"))
    hfm = []
    for f in range(DM // 128):
        pt = psT.tile([128, L], BF16, tag="psT")
        for i in range(n_tok_tiles):
            nc.tensor.transpose(pt[:, i * 128:(i + 1) * 128],
                                normed[i][:, f * 128:(f + 1) * 128],
                                ident_sb[:])
        ht = hfm_pool.tile([128, L], BF16, tag="hfm")
        nc.any.tensor_scalar(ht[:], pt[:], g_t[f][:], b_t[f][:],
                             ALU.mult, ALU.add)
        hfm.append(ht)
    return hfm


def build_nc(L=L_FULL):
    n_tok = L // 128
    CH = min(512, L)
    n_ch = L // CH  # token chunks for matmul moving dim

    nc = bacc.Bacc("TRN2", target_bir_lowering=False, debug=False,
                   num_devices=8)

    # ---- dram params ----
    def din(name, shape, dt=FP32):
        return nc.dram_tensor(name, shape, dt, kind="ExternalInput").ap()

    x_d = din("x", [L, DM])
    ln1_g = din("ln1_g", [DM, 1]); ln1_b = din("ln1_b", [DM, 1])
    ln2_g = din("ln2_g", [DM, 1]); ln2_b = din("ln2_b", [DM, 1])
    w_in_d = din("w_in", [DM, 2 * DIS], BF16)      # [xc cols | z cols]
    conv_w_d = din("conv_w", [DIS, DC])
    conv_b_d = din("conv_b", [DIS, 1])
    a_neg_d = din("a_neg", [DIS, NST])             # A = -exp(a_log) shard
    w_x_d = din("w_x", [DIS, RNK + 2 * NST], BF16)
    w_dt_d = din("w_dt", [RNK, DIS], BF16)
    b_dt_d = din("b_dt", [DIS, 1])
    d_skip_d = din("d_skip", [DIS, 1])
    w_out_d = din("w_out", [DIS, DM], BF16)
    w_ff1_d = din("w_ff1", [DM, 2 * FFS], BF16)    # [a cols | g cols]
    b_ff1_d = din("b_ff1", [2 * FFS, 1])
    w_ff2_d = din("w_ff2", [FFS, DM], BF16)
    ident_d = din("ident", [128, 128], BF16)

    x2_out = nc.dram_tensor("x2_out", [L, DM], FP32,
                            kind="ExternalOutput").ap()
    ffp_out = nc.dram_tensor("ffp_out", [L, DM], FP32,
                             kind="ExternalOutput").ap()

    with tile.TileContext(nc) as tc, ExitStack() as octx:
        dram = octx.enter_context(tc.tile_pool(name="dram", bufs=1,
                                               space="DRAM"))
        mm = octx.enter_context(tc.tile_pool(name="mm", bufs=4, space="PSUM"))
        const = octx.enter_context(tc.tile_pool(name="const", bufs=1))

        ident_sb = const.tile([128, 128], BF16, tag="ident")
        nc.sync.dma_start(out=ident_sb[:], in_=ident_d[:, :])

        # dram intermediates
        dbc_part = dram.tile([RNK + 2 * NST, L], FP32, tag="dbc_part")
        dbc_ar = dram.tile([RNK + 2 * NST, L], FP32, tag="dbc_ar")
        bbf_d = dram.tile([2 * NST, L], BF16, tag="bbf")
        z_dram = dram.tile([DIS, L], BF16, tag="z_dram")
        xc_dram = dram.tile([DIS, L], BF16, tag="xc_dram")
        m_part = dram.tile([L, DM], BF16, tag="m_part")
        m_ar = dram.tile([L, DM], BF16, tag="m_ar")

        groups = [[0, 1, 2, 3], [4, 5, 6, 7]]

        # Long-lived pools (phases 2-4) created first so shorter-lived
        # pools can pop in LIFO order before phase 5 reuses the space.
        pMain = octx.enter_context(ExitStack())
        wts = pMain.enter_context(tc.tile_pool(name="wts", bufs=1))
        sconst = pMain.enter_context(tc.tile_pool(name="sconst", bufs=1))
        act = pMain.enter_context(tc.tile_pool(name="act", bufs=1))
        dtp = pMain.enter_context(tc.tile_pool(name="dtp", bufs=1))
        dtxp = pMain.enter_context(tc.tile_pool(name="dtx", bufs=1))

        # ================= Phase 1: LN1 -> h_fm =================
        p12 = pMain.enter_context(ExitStack())
        hfm_pool = p12.enter_context(tc.tile_pool(name="hfm", bufs=8))
        with ExitStack() as p1:
            xload = p1.enter_context(tc.tile_pool(name="xload", bufs=2))
            xt_list = []
            for i in range(n_tok):
                xt = xload.tile([128, DM], FP32, tag="xt")
                nc.sync.dma_start(out=xt[:], in_=x_d[i * 128:(i + 1) * 128, :])
                xt_list.append(xt)
            # NOTE: xload bufs=3 but we keep refs; tiles with same tag share 3
            # slots -> must consume before reuse. LN consumes immediately, but
            # transposes need all normed tiles (not xt). OK.
            hfm = _layer_norm_stage(nc, tc, p1, xt_list, n_tok, ident_sb,
                                    ln1_g, ln1_b, hfm_pool, L, "ln1")

        # ================= Phase 2: in_proj, conv, dbc, dt =================
        p2 = p12
        w_in_sb = []
        for k in range(8):
            t = wts.tile([128, 2 * DIS], BF16, tag=f"w_in{k}")
            nc.sync.dma_start(out=t[:], in_=w_in_d[k * 128:(k + 1) * 128, :])
            w_in_sb.append(t)
        wx_sb = []
        for k in range(4):
            t = wts.tile([128, RNK + 2 * NST], BF16, tag=f"wx{k}")
            nc.sync.dma_start(out=t[:], in_=w_x_d[k * 128:(k + 1) * 128, :])
            wx_sb.append(t)
        wdt_sb = wts.tile([RNK, DIS], BF16, tag="wdt")
        nc.sync.dma_start(out=wdt_sb[:], in_=w_dt_d[:, :])

        cw_sb, cb_sb, a_sb, bdt_sb, dskip_sb = [], [], [], [], []
        for d in range(4):
            r = slice(d * 128, (d + 1) * 128)
            t = sconst.tile([128, DC], FP32, tag=f"cw{d}")
            nc.sync.dma_start(out=t[:], in_=conv_w_d[r, :]); cw_sb.append(t)
            t = sconst.tile([128, 1], FP32, tag=f"cb{d}")
            nc.sync.dma_start(out=t[:], in_=conv_b_d[r, :]); cb_sb.append(t)
            t = sconst.tile([128, NST], FP32, tag=f"a{d}")
            nc.sync.dma_start(out=t[:], in_=a_neg_d[r, :]); a_sb.append(t)
            t = sconst.tile([128, 1], FP32, tag=f"bdt{d}")
            nc.sync.dma_start(out=t[:], in_=b_dt_d[r, :]); bdt_sb.append(t)
            t = sconst.tile([128, 1], FP32, tag=f"dsk{d}")
            nc.sync.dma_start(out=t[:], in_=d_skip_d[r, :]); dskip_sb.append(t)

        xc_pad = []
        for d in range(4):
            t = act.tile([128, L + 3], BF16, tag=f"xcp{d}")
            nc.vector.memset(t[:, 0:3], 0.0)
            xc_pad.append(t)
        zsp = p12.enter_context(tc.tile_pool(name="zsp", bufs=3))

        # in_proj: out feature tile f (0..3 -> xc, 4..7 -> z)
        for f in range(8):
            for c in range(n_ch):
                ps = mm.tile([128, CH], FP32, tag="mm")
                for k in range(8):
                    nc.tensor.matmul(
                        ps[:], w_in_sb[k][:, f * 128:(f + 1) * 128],
                        hfm[k][:, c * CH:(c + 1) * CH],
                        start=(k == 0), stop=(k == 7))
                if f < 4:
                    nc.any.tensor_copy(
                        xc_pad[f][:, 3 + c * CH: 3 + (c + 1) * CH], ps[:])
                else:
                    zt = zsp.tile([128, CH], BF16, tag="zsp")
                    nc.any.tensor_copy(zt[:], ps[:])
                    nc.sync.dma_start(
                        out=z_dram[(f - 4) * 128:(f - 3) * 128,
                                   c * CH:(c + 1) * CH], in_=zt[:])

        # conv + silu (writes silu'd xc back into xc_pad[:, 3:3+L])
        cacc = p2.enter_context(tc.tile_pool(name="cacc", bufs=2))
        for d in range(4):
            acc = cacc.tile([128, L], BF16, tag="cacc")
            nc.vector.tensor_scalar_mul(acc[:], xc_pad[d][:, 0:L],
                                        cw_sb[d][:, 0:1])
            for j in range(1, DC):
                nc.vector.scalar_tensor_tensor(
                    acc[:], xc_pad[d][:, j:j + L], cw_sb[d][:, j:j + 1],
                    acc[:], ALU.mult, ALU.add)
            _act_silu(nc, cacc, xc_pad[d][:, 3:3 + L], acc[:],
                      bias=cb_sb[d][:])

        # dbc partial + AllReduce
        dbcp = p2.enter_context(tc.tile_pool(name="dbcp", bufs=1))
        dbc_sb = dbcp.tile([RNK + 2 * NST, L], FP32, tag="dbc")
        for c in range(n_ch):
            ps = mm.tile([RNK + 2 * NST, CH], FP32, tag="mm")
            for k in range(4):
                nc.tensor.matmul(ps[:], wx_sb[k][:],
                                 xc_pad[k][:, 3 + c * CH:3 + (c + 1) * CH],
                                 start=(k == 0), stop=(k == 3))
            nc.any.tensor_copy(dbc_sb[:, c * CH:(c + 1) * CH], ps[:])
        nc.gpsimd.dma_start(out=dbc_part[:], in_=dbc_sb[:])
        nc.gpsimd.collective_compute(
            "AllReduce", ALU.add, replica_groups=groups,
            ins=[dbc_part.opt()], outs=[dbc_ar.opt()])
        nc.sync.dma_start(out=dbc_sb[:], in_=dbc_ar[:])

        # dt_lo bf16 cast; B rows bf16 to dram for broadcast
        misc = p2.enter_context(tc.tile_pool(name="misc", bufs=1))
        dtlo = misc.tile([RNK, L], BF16, tag="dtlo")
        nc.any.tensor_copy(dtlo[:], dbc_sb[0:RNK, :])
        bbf = misc.tile([2 * NST, L], BF16, tag="bbf")
        nc.any.tensor_copy(bbf[:], dbc_sb[RNK:RNK + 2 * NST, :])
        nc.sync.dma_start(out=bbf_d[:], in_=bbf[:])

        # dt = softplus(dt_lo @ w_dt + b_dt)  (bf16 out)
        dt_sb = []
        for d in range(4):
            t = dtp.tile([128, L], BF16, tag=f"dt{d}")
            dt_sb.append(t)
            for c in range(n_ch):
                ps = mm.tile([128, CH], FP32, tag="mm")
                nc.tensor.matmul(ps[:], wdt_sb[:, d * 128:(d + 1) * 128],
                                 dtlo[:, c * CH:(c + 1) * CH],
                                 start=True, stop=True)
                _act_softplus(nc, dtp, t[:, c * CH:(c + 1) * CH], ps[:],
                              bdt_sb[d][:])

        # dtx = dt*xc (bf16) + spill xc for the gate stage, then free p12
        dtx_sb = []
        for d in range(4):
            dtx = dtxp.tile([128, L], BF16, tag=f"dtx{d}")
            nc.vector.tensor_mul(dtx[:], dt_sb[d][:], xc_pad[d][:, 3:3 + L])
            dtx_sb.append(dtx)
            nc.sync.dma_start(out=xc_dram[d * 128:(d + 1) * 128, :],
                              in_=xc_pad[d][:, 3:3 + L])
        p12.close()

        # ================= Phase 3: scan + gate =================
        p3 = pMain.enter_context(ExitStack())
        sc = p3.enter_context(tc.tile_pool(name="scan", bufs=2))
        bcp = p3.enter_context(tc.tile_pool(name="bcast", bufs=2))
        yp = p3.enter_context(tc.tile_pool(name="y", bufs=1))
        gp = p3.enter_context(tc.tile_pool(name="gated", bufs=4))
        y_sb = []
        for d in range(4):
            y_t = yp.tile([128, L], FP32, tag=f"y{d}")
            y_sb.append(y_t)
        for n in range(NST):
            bcB = bcp.tile([128, L], BF16, tag="bcB")
            nc.sync.dma_start(
                out=bcB[:], in_=bbf_d[n:n + 1, :].broadcast_to([128, L]))
            bcC = bcp.tile([128, L], BF16, tag="bcC")
            nc.sync.dma_start(
                out=bcC[:],
                in_=bbf_d[NST + n:NST + n + 1, :].broadcast_to([128, L]))
            for d in range(4):
                dA = sc.tile([128, L], BF16, tag="dA")
                nc.scalar.activation(dA[:], dt_sb[d][:], ACTF.Exp,
                                     scale=a_sb[d][:, n:n + 1])
                dB = sc.tile([128, L], BF16, tag="dB")
                nc.vector.tensor_mul(dB[:], dtx_sb[d][:], bcB[:])
                # h (in-place over dB): state = dA*state + dB
                nc.vector.tensor_tensor_scan(dB[:], dA[:], dB[:], 0.0,
                                             ALU.mult, ALU.add)
                if n == 0:
                    nc.gpsimd.tensor_mul(y_sb[d][:], dB[:], bcC[:])
                else:
                    nc.gpsimd.tensor_mul(dA[:], dB[:], bcC[:])  # ch into dA
                    nc.vector.tensor_add(y_sb[d][:], y_sb[d][:], dA[:])
        # gate: gated = (y + d_skip*xc) * silu(z)   (z/xc reloaded from dram)
        gated_sb = []
        zld = p3.enter_context(tc.tile_pool(name="zld", bufs=1))
        for d in range(4):
            r = slice(d * 128, (d + 1) * 128)
            zt = zld.tile([128, L], BF16, tag="zt")
            nc.sync.dma_start(out=zt[:], in_=z_dram[r, :])
            xct = zld.tile([128, L], BF16, tag="xct")
            nc.sync.dma_start(out=xct[:], in_=xc_dram[r, :])
            sz = zld.tile([128, L], FP32, tag="sz")
            _act_silu(nc, sc, sz[:], zt[:])
            tmp = zld.tile([128, L], FP32, tag="gtmp")
            nc.vector.tensor_scalar_mul(tmp[:], xct[:], dskip_sb[d][:])
            nc.vector.tensor_add(y_sb[d][:], y_sb[d][:], tmp[:])
            gt = gp.tile([128, L], BF16, tag="gated")
            nc.vector.tensor_mul(gt[:], y_sb[d][:], sz[:])
            gated_sb.append(gt)

        # ================= Phase 4: out_proj + AR2 =================
        wout_sb = []
        for k in range(4):
            t = wts.tile([128, DM], BF16, tag=f"wout{k}")
            nc.sync.dma_start(out=t[:], in_=w_out_d[k * 128:(k + 1) * 128, :])
            wout_sb.append(t)
        mp_pool = p3.enter_context(tc.tile_pool(name="mp", bufs=2))
        tiles_per_cc = max(1, n_tok // 4)
        for i in range(n_tok):
            mp = mp_pool.tile([128, DM], BF16, tag="mp")
            for nchk in range(2):
                ps = mm.tile([128, 512], FP32, tag="mm")
                for k in range(4):
                    nc.tensor.matmul(
                        ps[:], gated_sb[k][:, i * 128:(i + 1) * 128],
                        wout_sb[k][:, nchk * 512:(nchk + 1) * 512],
                        start=(k == 0), stop=(k == 3))
                nc.any.tensor_copy(mp[:, nchk * 512:(nchk + 1) * 512], ps[:])
            nc.sync.dma_start(out=m_part[i * 128:(i + 1) * 128, :], in_=mp[:])
            if (i + 1) % tiles_per_cc == 0:
                r0 = (i + 1 - tiles_per_cc) * 128
                r1 = (i + 1) * 128
                nc.gpsimd.collective_compute(
                    "AllReduce", ALU.add, replica_groups=groups,
                    ins=[m_part[r0:r1, :].opt()],
                    outs=[m_ar[r0:r1, :].opt()])
        p3.close()
        pMain.close()

        # ================= Phase 5: residual + LN2 =================
        h2fm_pool = octx.enter_context(tc.tile_pool(name="h2fm", bufs=8))
        with ExitStack() as p5:
            x2p = p5.enter_context(tc.tile_pool(name="x2", bufs=4))
            ld = p5.enter_context(tc.tile_pool(name="ld5", bufs=3))
            x2_list = []
            for i in range(n_tok):
                r = slice(i * 128, (i + 1) * 128)
                xt = ld.tile([128, DM], FP32, tag="xr")
                nc.sync.dma_start(out=xt[:], in_=x_d[r, :])
                mt = ld.tile([128, DM], BF16, tag="mr")
                nc.sync.dma_start(out=mt[:], in_=m_ar[r, :])
                mtf = ld.tile([128, DM], FP32, tag="mrf")
                nc.any.tensor_copy(mtf[:], mt[:])
                x2 = x2p.tile([128, DM], FP32, tag="x2")
                nc.vector.tensor_add(x2[:], xt[:], mtf[:])
                nc.sync.dma_start(out=x2_out[r, :], in_=x2[:])
                x2_list.append(x2)
            h2fm = _layer_norm_stage(nc, tc, p5, x2_list, n_tok, ident_sb,
                                     ln2_g, ln2_b, h2fm_pool, L, "ln2")

        # ================= Phase 6: FF =================
        with ExitStack() as p6:
            wf = p6.enter_context(tc.tile_pool(name="wf", bufs=1))
            wff1_sb = []
            for k in range(8):
                t = wf.tile([128, 2 * FFS], BF16, tag=f"wff1{k}")
                nc.sync.dma_start(out=t[:],
                                  in_=w_ff1_d[k * 128:(k + 1) * 128, :])
                wff1_sb.append(t)
            wff2_sb = []
            for k in range(8):
                t = wf.tile([128, DM], BF16, tag=f"wff2{k}")
                nc.sync.dma_start(out=t[:],
                                  in_=w_ff2_d[k * 128:(k + 1) * 128, :])
                wff2_sb.append(t)
            bf1 = []
            for j in range(2 * FFS // 128):
                t = wf.tile([128, 1], FP32, tag=f"bf1_{j}")
                nc.sync.dma_start(out=t[:], in_=b_ff1_d[j * 128:(j + 1) * 128, :])
                bf1.append(t)

            agp = p6.enter_context(tc.tile_pool(name="ag", bufs=8))
            tmp6 = p6.enter_context(tc.tile_pool(name="tmp6", bufs=4))
            ag_sb = []
            for fa in range(8):
                agt = agp.tile([128, L], BF16, tag="ag")
                ag_sb.append(agt)
                for c in range(n_ch):
                    psA = mm.tile([128, CH], FP32, tag="mm")
                    for k in range(8):
                        nc.tensor.matmul(
                            psA[:], wff1_sb[k][:, fa * 128:(fa + 1) * 128],
                            h2fm[k][:, c * CH:(c + 1) * CH],
                            start=(k == 0), stop=(k == 7))
                    psG = mm.tile([128, CH], FP32, tag="mm")
                    for k in range(8):
                        nc.tensor.matmul(
                            psG[:],
                            wff1_sb[k][:, FFS + fa * 128:FFS + (fa + 1) * 128],
                            h2fm[k][:, c * CH:(c + 1) * CH],
                            start=(k == 0), stop=(k == 7))
                    aa = tmp6.tile([128, CH], BF16, tag="aa")
                    nc.scalar.activation(aa[:], psA[:], ACTF.Identity,
                                         bias=bf1[fa][:])
                    gg = tmp6.tile([128, CH], BF16, tag="gg")
                    _act_gelu(nc, tmp6, gg[:], psG[:],
                              bf1[FFS // 128 + fa][:])
                    nc.vector.tensor_mul(agt[:, c * CH:(c + 1) * CH],
                                         aa[:], gg[:])

            ffo = p6.enter_context(tc.tile_pool(name="ffo", bufs=2))
            for i in range(n_tok):
                fo = ffo.tile([128, DM], FP32, tag="fo")
                for nchk in range(2):
                    ps = mm.tile([128, 512], FP32, tag="mm")
                    for k in range(8):
                        nc.tensor.matmul(
                            ps[:], ag_sb[k][:, i * 128:(i + 1) * 128],
                            wff2_sb[k][:, nchk * 512:(nchk + 1) * 512],
                            start=(k == 0), stop=(k == 7))
                    nc.any.tensor_copy(fo[:, nchk * 512:(nchk + 1) * 512],
                                       ps[:])
                nc.sync.dma_start(out=ffp_out[i * 128:(i + 1) * 128, :],
                                  in_=fo[:])
    nc.compile()
    return nc


_NC_CACHE = {}


def _get_nc(L=L_FULL):
    if L not in _NC_CACHE:
        _NC_CACHE[L] = build_nc(L)
    return _NC_CACHE[L]


def make_in_maps(x, ln1_g, ln1_b, w_in, conv_w, conv_b, w_x, w_dt, b_dt,
                 a_log, d_skip, w_out, ln2_g, ln2_b, w_ff1, b_ff1, w_ff2,
                 b_ff2):
    x = np.asarray(x, np.float32)
    f32 = lambda a: np.ascontiguousarray(np.asarray(a, np.float32))
    bf = lambda a: np.ascontiguousarray(np.asarray(a, np.float32)).astype(NPBF16)
    ident = np.eye(128, dtype=np.float32).astype(NPBF16)
    a_neg = -np.exp(np.asarray(a_log, np.float32))
    in_maps = []
    for c in range(8):
        b, s = c // 4, c % 4
        ds = slice(s * DIS, (s + 1) * DIS)
        fs = slice(s * FFS, (s + 1) * FFS)
        gs = slice(FFI + s * FFS, FFI + (s + 1) * FFS)
        in_maps.append(dict(
            x=f32(x[b]),
            ln1_g=f32(ln1_g).reshape(DM, 1), ln1_b=f32(ln1_b).reshape(DM, 1),
            ln2_g=f32(ln2_g).reshape(DM, 1), ln2_b=f32(ln2_b).reshape(DM, 1),
            w_in=bf(np.concatenate(
                [w_in[:, s * DIS:(s + 1) * DIS],
                 w_in[:, DI + s * DIS:DI + (s + 1) * DIS]], axis=1)),
            conv_w=f32(conv_w[ds]), conv_b=f32(conv_b[ds]).reshape(DIS, 1),
            a_neg=f32(a_neg[ds]),
            w_x=bf(w_x[ds]), w_dt=bf(w_dt[:, ds]),
            b_dt=f32(b_dt[ds]).reshape(DIS, 1),
            d_skip=f32(d_skip[ds]).reshape(DIS, 1),
            w_out=bf(w_out[ds]),
            w_ff1=bf(np.concatenate([w_ff1[:, fs], w_ff1[:, gs]], axis=1)),
            b_ff1=f32(np.concatenate(
                [b_ff1[s * FFS:(s + 1) * FFS],
                 b_ff1[FFI + s * FFS:FFI + (s + 1) * FFS]])).reshape(2 * FFS, 1),
            w_ff2=bf(w_ff2[fs]),
            ident=ident,
        ))
    return in_maps


def combine_outputs(results, b_ff2, L=L_FULL):
    out = np.zeros((B_FULL, L, DM), np.float32)
    bff2 = np.asarray(b_ff2, np.float32)
    for b in range(B_FULL):
        acc = results[4 * b]["x2_out"].astype(np.float32).copy()
        for s in range(4):
            acc += results[4 * b + s]["ffp_out"].astype(np.float32)
        out[b] = acc + bff2[None, :]
    return out


def kernel(**inputs):
    nc = _get_nc(L_FULL)
    in_maps = make_in_maps(
        inputs["x"], inputs["ln1_g"], inputs["ln1_b"], inputs["w_in"],
        inputs["conv_w"], inputs["conv_b"], inputs["w_x"], inputs["w_dt"],
        inputs["b_dt"], inputs["a_log"], inputs["d_skip"], inputs["w_out"],
        inputs["ln2_g"], inputs["ln2_b"], inputs["w_ff1"], inputs["b_ff1"],
        inputs["w_ff2"], inputs["b_ff2"])
    res = run_bass_kernel_spmd(nc, in_maps, core_ids=list(range(8)))
    return combine_outputs(res.results, inputs["b_ff2"], L_FULL)


# revision 24
# speedup vs baseline: 43.7515x; 1.6649x over previous
"""Trainium2 Bass kernel for BasicMambaBlock (B=2, L=2048, d_model=1024).

Sharding: 8 cores = 2 batch groups x 4 TP shards.
Per core: d_inner shard = 512 channels, FF shard = 1024 a-cols + 1024 g-cols.
Feature-major (transposed) activation layout for the matmul chain; Mamba
recurrence via tensor_tensor_scan (channels on partitions, time on free dim).
Two in-group AllReduces (dbc partials (96,L) and out_proj partials (L,1024)).
Final FF partial sums + residual combined on host.
"""
import sys

sys.path.insert(0, "/opt/trn_rl_repo")

import numpy as np
import ml_dtypes
from contextlib import ExitStack

import concourse.bass as bass
import concourse.tile as tile
from concourse import bacc, mybir
from concourse.bass_utils import run_bass_kernel_spmd

FP32 = mybir.dt.float32
BF16 = mybir.dt.bfloat16
ALU = mybir.AluOpType
ACTF = mybir.ActivationFunctionType
NPBF16 = ml_dtypes.bfloat16

DM = 1024          # d_model
DI = 2048          # d_inner (global)
DIS = DI // 4      # 512 per-core d_inner shard
NST = 16           # d_state
RNK = 64           # dt_rank
DC = 4             # conv width
FFI = 4096         # ff inner (global)
FFS = FFI // 4     # 1024 per-core ff shard
EPS = 1e-5
L_FULL = 2048
B_FULL = 2

# When True, emit compositions of sim-supported ACT funcs instead of
# Silu/Softplus/Gelu (the CoreSim interpreter lacks those LUTs).
SIM_SAFE = False


def _act_silu(nc, scr, out, in_, bias=None):
    """out = silu(in_ + bias)."""
    if not SIM_SAFE:
        nc.scalar.activation(out, in_, ACTF.Silu,
                             bias=(bias if bias is not None else 0.0))
        return
    shape = [in_.shape[0], in_.free_size()]
    v = scr.tile(shape, FP32, tag="silu_v")
    nc.scalar.activation(v[:], in_, ACTF.Identity,
                         bias=(bias if bias is not None else 0.0))
    s = scr.tile(shape, FP32, tag="silu_s")
    nc.scalar.activation(s[:], v[:], ACTF.Sigmoid)
    nc.vector.tensor_mul(out, v[:], s[:])


def _act_softplus(nc, scr, out, in_, bias):
    """out = softplus(in_ + bias) = ln(1 + exp(in_ + bias)).

    Composed from Exp+Ln (same ACT table set as the scan's Exp) because
    this walrus build has no Softplus LUT set."""
    shape = [in_.shape[0], in_.free_size()]
    e = scr.tile(shape, FP32, tag="sp_e")
    nc.scalar.activation(e[:], in_, ACTF.Exp, bias=bias)
    nc.vector.tensor_scalar_add(e[:], e[:], 1.0)
    nc.scalar.activation(out, e[:], ACTF.Ln)


def _act_gelu(nc, scr, out, in_, bias):
    """out = gelu_tanh(in_ + bias)."""
    if not SIM_SAFE:
        nc.scalar.activation(out, in_, ACTF.Gelu_apprx_tanh, bias=bias)
        return
    shape = [in_.shape[0], in_.free_size()]
    v = scr.tile(shape, FP32, tag="ge_v")
    nc.scalar.activation(v[:], in_, ACTF.Identity, bias=bias)
    v2 = scr.tile(shape, FP32, tag="ge_v2")
    nc.scalar.activation(v2[:], v[:], ACTF.Square)
    v3 = scr.tile(shape, FP32, tag="ge_v3")
    nc.vector.tensor_mul(v3[:], v2[:], v[:])
    u = scr.tile(shape, FP32, tag="ge_u")
    nc.vector.scalar_tensor_tensor(u[:], v3[:], 0.044715, v[:],
                                   ALU.mult, ALU.add)
    w = scr.tile(shape, FP32, tag="ge_w")
    nc.scalar.activation(w[:], u[:], ACTF.Tanh, scale=0.7978845608028654)
    nc.vector.tensor_scalar(w[:], w[:], 1.0, 0.5, ALU.add, ALU.mult)
    nc.vector.tensor_mul(out, v[:], w[:])


def _layer_norm_stage(nc, tc, ctx, src_tiles, n_tok_tiles, ident_sb, g_ap, b_ap,
                      hfm_pool, L, name):
    """Token-major LN on src_tiles (list of (128, DM) fp32 sbuf tiles) ->
    feature-major bf16 tiles (8 x (128, L)), with g/b applied per-partition
    after the transpose. Returns list of 8 hfm tiles."""
    stat = ctx.enter_context(tc.tile_pool(name=f"{name}_stat", bufs=4))
    scr = ctx.enter_context(tc.tile_pool(name=f"{name}_scr", bufs=1))
    nrm = ctx.enter_context(tc.tile_pool(name=f"{name}_nrm", bufs=n_tok_tiles))
    gsb = ctx.enter_context(tc.tile_pool(name=f"{name}_gb", bufs=1))

    # g/b per-feature: 8 x (128,1) tiles
    g_t, b_t = [], []
    for f in range(DM // 128):
        t = gsb.tile([128, 1], FP32, tag=f"g{f}")
        nc.sync.dma_start(out=t[:], in_=g_ap[f * 128:(f + 1) * 128, :])
        g_t.append(t)
        t = gsb.tile([128, 1], FP32, tag=f"b{f}")
        nc.sync.dma_start(out=t[:], in_=b_ap[f * 128:(f + 1) * 128, :])
        b_t.append(t)

    eps_t = gsb.tile([128, 1], FP32, tag="eps")
    nc.vector.memset(eps_t[:], EPS)

    normed = []
    for i in range(n_tok_tiles):
        xt = src_tiles[i]
        s1 = stat.tile([128, 1], FP32, tag="s1")
        nc.vector.tensor_reduce(s1[:], xt[:], mybir.AxisListType.X, ALU.add)
        sq = scr.tile([128, DM], FP32, tag="sq")
        s2 = stat.tile([128, 1], FP32, tag="s2")
        nc.scalar.activation(sq[:], xt[:], ACTF.Square, accum_out=s2[:])
        mu = stat.tile([128, 1], FP32, tag="mu")
        nc.vector.tensor_scalar_mul(mu[:], s1[:], 1.0 / DM)
        var = stat.tile([128, 1], FP32, tag="var")
        # var = s2/DM - mu^2
        musq = stat.tile([128, 1], FP32, tag="musq")
        nc.vector.tensor_mul(musq[:], mu[:], mu[:])
        nc.vector.tensor_scalar(var[:], s2[:], 1.0 / DM, None, ALU.mult)
        nc.vector.tensor_sub(var[:], var[:], musq[:])
        lv = stat.tile([128, 1], FP32, tag="lv")
        nc.scalar.activation(lv[:], var[:], ACTF.Ln, bias=eps_t[:])
        rstd = stat.tile([128, 1], FP32, tag="rstd")
        nc.scalar.activation(rstd[:], lv[:], ACTF.Exp, scale=-0.5)
        nt = nrm.tile([128, DM], BF16, tag="normed")
        nc.vector.tensor_scalar(nt[:], xt[:], mu[:], rstd[:],
                                ALU.subtract, ALU.mult)
        normed.append(nt)

    # transpose to feature-major; fuse g/b at PSUM evacuation
    psT = ctx.enter_context(tc.tile_pool(name=f"{name}_psT", bufs=2,
                                         space="PSUM"))
    hfm = []
    for f in range(DM // 128):
        pt = psT.tile([128, L], BF16, tag="psT")
        for i in range(n_tok_tiles):
            nc.tensor.transpose(pt[:, i * 128:(i + 1) * 128],
                                normed[i][:, f * 128:(f + 1) * 128],
                                ident_sb[:])
        ht = hfm_pool.tile([128, L], BF16, tag="hfm")
        nc.any.tensor_scalar(ht[:], pt[:], g_t[f][:], b_t[f][:],
                             ALU.mult, ALU.add)
        hfm.append(ht)
    return hfm


def build_nc(L=L_FULL):
    n_tok = L // 128
    CH = min(512, L)
    n_ch = L // CH  # token chunks for matmul moving dim

    nc = bacc.Bacc("TRN2", target_bir_lowering=False, debug=False,
                   num_devices=8)

    # ---- dram params ----
    def din(name, shape, dt=FP32):
        return nc.dram_tensor(name, shape, dt, kind="ExternalInput").ap()

    x_d = din("x", [L, DM])
    ln1_g = din("ln1_g", [DM, 1]); ln1_b = din("ln1_b", [DM, 1])
    ln2_g = din("ln2_g", [DM, 1]); ln2_b = din("ln2_b", [DM, 1])
    w_in_d = din("w_in", [DM, 2 * DIS], BF16)      # [xc cols | z cols]
    conv_w_d = din("conv_w", [DIS, DC])
    conv_b_d = din("conv_b", [DIS, 1])
    a_neg_d = din("a_neg", [DIS, NST])             # A = -exp(a_log) shard
    w_x_d = din("w_x", [DIS, RNK + 2 * NST], BF16)
    w_dt_d = din("w_dt", [RNK, DIS], BF16)
    b_dt_d = din("b_dt", [DIS, 1])
    d_skip_d = din("d_skip", [DIS, 1])
    w_out_d = din("w_out", [DIS, DM], BF16)
    w_ff1_d = din("w_ff1", [DM, 2 * FFS], BF16)    # [a cols | g cols]
    b_ff1_d = din("b_ff1", [2 * FFS, 1])
    w_ff2_d = din("w_ff2", [FFS, DM], BF16)
    ident_d = din("ident", [128, 128], BF16)

    x2_out = nc.dram_tensor("x2_out", [L, DM], FP32,
                            kind="ExternalOutput").ap()
    ffp_out = nc.dram_tensor("ffp_out", [L, DM], FP32,
                             kind="ExternalOutput").ap()

    with tile.TileContext(nc) as tc, ExitStack() as octx:
        dram = octx.enter_context(tc.tile_pool(name="dram", bufs=1,
                                               space="DRAM"))
        mm = octx.enter_context(tc.tile_pool(name="mm", bufs=4, space="PSUM"))
        const = octx.enter_context(tc.tile_pool(name="const", bufs=1))

        ident_sb = const.tile([128, 128], BF16, tag="ident")
        nc.sync.dma_start(out=ident_sb[:], in_=ident_d[:, :])

        # dram intermediates
        dbc_part = dram.tile([RNK + 2 * NST, L], FP32, tag="dbc_part")
        dbc_ar = dram.tile([RNK + 2 * NST, L], FP32, tag="dbc_ar")
        bbf_d = dram.tile([2 * NST, L], BF16, tag="bbf")
        z_dram = dram.tile([DIS, L], BF16, tag="z_dram")
        xc_dram = dram.tile([DIS, L], BF16, tag="xc_dram")
        m_part = dram.tile([L, DM], BF16, tag="m_part")
        m_ar = dram.tile([L, DM], BF16, tag="m_ar")

        groups = [[0, 1, 2, 3], [4, 5, 6, 7]]

        # Long-lived pools (phases 2-4) created first so shorter-lived
        # pools can pop in LIFO order before phase 5 reuses the space.
        pMain = octx.enter_context(ExitStack())
        wts = pMain.enter_context(tc.tile_pool(name="wts", bufs=1))
        sconst = pMain.enter_context(tc.tile_pool(name="sconst", bufs=1))
        act = pMain.enter_context(tc.tile_pool(name="act", bufs=1))
        dtp = pMain.enter_context(tc.tile_pool(name="dtp", bufs=1))
        dtxp = pMain.enter_context(tc.tile_pool(name="dtx", bufs=1))

        # ================= Phase 1: LN1 -> h_fm =================
        p12 = pMain.enter_context(ExitStack())
        hfm_pool = p12.enter_context(tc.tile_pool(name="hfm", bufs=8))
        with ExitStack() as p1:
            xload = p1.enter_context(tc.tile_pool(name="xload", bufs=2))
            xt_list = []
            for i in range(n_tok):
                xt = xload.tile([128, DM], FP32, tag="xt")
                nc.sync.dma_start(out=xt[:], in_=x_d[i * 128:(i + 1) * 128, :])
                xt_list.append(xt)
            # NOTE: xload bufs=3 but we keep refs; tiles with same tag share 3
            # slots -> must consume before reuse. LN consumes immediately, but
            # transposes need all normed tiles (not xt). OK.
            hfm = _layer_norm_stage(nc, tc, p1, xt_list, n_tok, ident_sb,
                                    ln1_g, ln1_b, hfm_pool, L, "ln1")

        # ================= Phase 2: in_proj, conv, dbc, dt =================
        p2 = p12
        w_in_sb = []
        for k in range(8):
            t = wts.tile([128, 2 * DIS], BF16, tag=f"w_in{k}")
            nc.sync.dma_start(out=t[:], in_=w_in_d[k * 128:(k + 1) * 128, :])
            w_in_sb.append(t)
        wx_sb = []
        for k in range(4):
            t = wts.tile([128, RNK + 2 * NST], BF16, tag=f"wx{k}")
            nc.sync.dma_start(out=t[:], in_=w_x_d[k * 128:(k + 1) * 128, :])
            wx_sb.append(t)
        wdt_sb = wts.tile([RNK, DIS], BF16, tag="wdt")
        nc.sync.dma_start(out=wdt_sb[:], in_=w_dt_d[:, :])

        cw_sb, cb_sb, a_sb, bdt_sb, dskip_sb = [], [], [], [], []
        for d in range(4):
            r = slice(d * 128, (d + 1) * 128)
            t = sconst.tile([128, DC], FP32, tag=f"cw{d}")
            nc.sync.dma_start(out=t[:], in_=conv_w_d[r, :]); cw_sb.append(t)
            t = sconst.tile([128, 1], FP32, tag=f"cb{d}")
            nc.sync.dma_start(out=t[:], in_=conv_b_d[r, :]); cb_sb.append(t)
            t = sconst.tile([128, NST], FP32, tag=f"a{d}")
            nc.sync.dma_start(out=t[:], in_=a_neg_d[r, :]); a_sb.append(t)
            t = sconst.tile([128, 1], FP32, tag=f"bdt{d}")
            nc.sync.dma_start(out=t[:], in_=b_dt_d[r, :]); bdt_sb.append(t)
            t = sconst.tile([128, 1], FP32, tag=f"dsk{d}")
            nc.sync.dma_start(out=t[:], in_=d_skip_d[r, :]); dskip_sb.append(t)

        xc_pad = []
        for d in range(4):
            t = act.tile([128, L + 3], BF16, tag=f"xcp{d}")
            nc.vector.memset(t[:, 0:3], 0.0)
            xc_pad.append(t)
        zsp = p12.enter_context(tc.tile_pool(name="zsp", bufs=3))

        # in_proj: out feature tile f (0..3 -> xc, 4..7 -> z)
        for f in range(8):
            for c in range(n_ch):
                ps = mm.tile([128, CH], FP32, tag="mm")
                for k in range(8):
                    nc.tensor.matmul(
                        ps[:], w_in_sb[k][:, f * 128:(f + 1) * 128],
                        hfm[k][:, c * CH:(c + 1) * CH],
                        start=(k == 0), stop=(k == 7))
                if f < 4:
                    nc.any.tensor_copy(
                        xc_pad[f][:, 3 + c * CH: 3 + (c + 1) * CH], ps[:])
                else:
                    zt = zsp.tile([128, CH], BF16, tag="zsp")
                    nc.any.tensor_copy(zt[:], ps[:])
                    nc.sync.dma_start(
                        out=z_dram[(f - 4) * 128:(f - 3) * 128,
                                   c * CH:(c + 1) * CH], in_=zt[:])

        # conv + silu (writes silu'd xc back into xc_pad[:, 3:3+L])
        cacc = p2.enter_context(tc.tile_pool(name="cacc", bufs=2))
        for d in range(4):
            acc = cacc.tile([128, L], BF16, tag="cacc")
            nc.vector.tensor_scalar_mul(acc[:], xc_pad[d][:, 0:L],
                                        cw_sb[d][:, 0:1])
            for j in range(1, DC):
                nc.vector.scalar_tensor_tensor(
                    acc[:], xc_pad[d][:, j:j + L], cw_sb[d][:, j:j + 1],
                    acc[:], ALU.mult, ALU.add)
            _act_silu(nc, cacc, xc_pad[d][:, 3:3 + L], acc[:],
                      bias=cb_sb[d][:])

        # dbc partial + AllReduce
        dbcp = p2.enter_context(tc.tile_pool(name="dbcp", bufs=1))
        dbc_sb = dbcp.tile([RNK + 2 * NST, L], FP32, tag="dbc")
        for c in range(n_ch):
            ps = mm.tile([RNK + 2 * NST, CH], FP32, tag="mm")
            for k in range(4):
                nc.tensor.matmul(ps[:], wx_sb[k][:],
                                 xc_pad[k][:, 3 + c * CH:3 + (c + 1) * CH],
                                 start=(k == 0), stop=(k == 3))
            nc.any.tensor_copy(dbc_sb[:, c * CH:(c + 1) * CH], ps[:])
        nc.gpsimd.dma_start(out=dbc_part[:], in_=dbc_sb[:])
        nc.gpsimd.collective_compute(
            "AllReduce", ALU.add, replica_groups=groups,
            ins=[dbc_part.opt()], outs=[dbc_ar.opt()])
        nc.sync.dma_start(out=dbc_sb[:], in_=dbc_ar[:])

        # dt_lo bf16 cast; B rows bf16 to dram for broadcast
        misc = p2.enter_context(tc.tile_pool(name="misc", bufs=1))
        dtlo = misc.tile([RNK, L], BF16, tag="dtlo")
        nc.any.tensor_copy(dtlo[:], dbc_sb[0:RNK, :])
        bbf = misc.tile([2 * NST, L], BF16, tag="bbf")
        nc.any.tensor_copy(bbf[:], dbc_sb[RNK:RNK + 2 * NST, :])
        nc.sync.dma_start(out=bbf_d[:], in_=bbf[:])

        # dt = softplus(dt_lo @ w_dt + b_dt)  (bf16 out)
        dt_sb = []
        for d in range(4):
            t = dtp.tile([128, L], BF16, tag=f"dt{d}")
            dt_sb.append(t)
            for c in range(n_ch):
                ps = mm.tile([128, CH], FP32, tag="mm")
                nc.tensor.matmul(ps[:], wdt_sb[:, d * 128:(d + 1) * 128],
                                 dtlo[:, c * CH:(c + 1) * CH],
                                 start=True, stop=True)
                _act_softplus(nc, dtp, t[:, c * CH:(c + 1) * CH], ps[:],
                              bdt_sb[d][:])

        # dtx = dt*xc (bf16) + spill xc for the gate stage, then free p12
        dtx_sb = []
        for d in range(4):
            dtx = dtxp.tile([128, L], BF16, tag=f"dtx{d}")
            nc.vector.tensor_mul(dtx[:], dt_sb[d][:], xc_pad[d][:, 3:3 + L])
            dtx_sb.append(dtx)
            nc.sync.dma_start(out=xc_dram[d * 128:(d + 1) * 128, :],
                              in_=xc_pad[d][:, 3:3 + L])
        p12.close()

        # ================= Phase 3: scan + gate =================
        p3 = pMain.enter_context(ExitStack())
        mm2 = p3.enter_context(tc.tile_pool(name="mm2", bufs=4, space="PSUM"))
        sc = p3.enter_context(tc.tile_pool(name="scan", bufs=2))
        bcp = p3.enter_context(tc.tile_pool(name="bcast", bufs=2))
        yp = p3.enter_context(tc.tile_pool(name="y", bufs=1))
        gp = p3.enter_context(tc.tile_pool(name="gated", bufs=4))
        y_sb = []
        for d in range(4):
            y_t = yp.tile([128, L], FP32, tag=f"y{d}")
            y_sb.append(y_t)
        for n in range(NST):
            bcB = bcp.tile([128, L], BF16, tag="bcB")
            nc.sync.dma_start(
                out=bcB[:], in_=bbf_d[n:n + 1, :].broadcast_to([128, L]))
            bcC = bcp.tile([128, L], BF16, tag="bcC")
            nc.sync.dma_start(
                out=bcC[:],
                in_=bbf_d[NST + n:NST + n + 1, :].broadcast_to([128, L]))
            for d in range(4):
                dA = sc.tile([128, L], BF16, tag="dA")
                nc.scalar.activation(dA[:], dt_sb[d][:], ACTF.Exp,
                                     scale=a_sb[d][:, n:n + 1])
                dB = sc.tile([128, L], BF16, tag="dB")
                nc.vector.tensor_mul(dB[:], dtx_sb[d][:], bcB[:])
                # h (in-place over dB): state = dA*state + dB
                nc.vector.tensor_tensor_scan(dB[:], dA[:], dB[:], 0.0,
                                             ALU.mult, ALU.add)
                if n == 0:
                    nc.gpsimd.tensor_mul(y_sb[d][:], dB[:], bcC[:])
                else:
                    nc.gpsimd.tensor_mul(dA[:], dB[:], bcC[:])  # ch into dA
                    nc.vector.tensor_add(y_sb[d][:], y_sb[d][:], dA[:])
        # gate: gated = (y + d_skip*xc) * silu(z)   (z/xc reloaded from dram)
        gated_sb = []
        zld = p3.enter_context(tc.tile_pool(name="zld", bufs=1))
        for d in range(4):
            r = slice(d * 128, (d + 1) * 128)
            zt = zld.tile([128, L], BF16, tag="zt")
            nc.sync.dma_start(out=zt[:], in_=z_dram[r, :])
            xct = zld.tile([128, L], BF16, tag="xct")
            nc.sync.dma_start(out=xct[:], in_=xc_dram[r, :])
            sz = zld.tile([128, L], FP32, tag="sz")
            _act_silu(nc, sc, sz[:], zt[:])
            tmp = zld.tile([128, L], FP32, tag="gtmp")
            nc.vector.tensor_scalar_mul(tmp[:], xct[:], dskip_sb[d][:])
            nc.vector.tensor_add(y_sb[d][:], y_sb[d][:], tmp[:])
            gt = gp.tile([128, L], BF16, tag="gated")
            nc.vector.tensor_mul(gt[:], y_sb[d][:], sz[:])
            gated_sb.append(gt)

        # ================= Phase 4: out_proj + AR2 =================
        wout_sb = []
        for k in range(4):
            t = wts.tile([128, DM], BF16, tag=f"wout{k}")
            nc.sync.dma_start(out=t[:], in_=w_out_d[k * 128:(k + 1) * 128, :])
            wout_sb.append(t)
        mp_pool = p3.enter_context(tc.tile_pool(name="mp", bufs=2))
        tiles_per_cc = max(1, n_tok // 4)
        for i in range(n_tok):
            mp = mp_pool.tile([128, DM], BF16, tag="mp")
            for nchk in range(2):
                pool = mm if nchk == 0 else mm2
                ps = pool.tile([128, 512], FP32, tag=pool.name)
                for k in range(4):
                    nc.tensor.matmul(
                        ps[:], gated_sb[k][:, i * 128:(i + 1) * 128],
                        wout_sb[k][:, nchk * 512:(nchk + 1) * 512],
                        start=(k == 0), stop=(k == 3))
                nc.any.tensor_copy(mp[:, nchk * 512:(nchk + 1) * 512], ps[:])
            nc.sync.dma_start(out=m_part[i * 128:(i + 1) * 128, :], in_=mp[:])
            if (i + 1) % tiles_per_cc == 0:
                r0 = (i + 1 - tiles_per_cc) * 128
                r1 = (i + 1) * 128
                nc.gpsimd.collective_compute(
                    "AllReduce", ALU.add, replica_groups=groups,
                    ins=[m_part[r0:r1, :].opt()],
                    outs=[m_ar[r0:r1, :].opt()])
        p3.close()
        pMain.close()

        # ================= Phase 5: residual + LN2 =================
        h2fm_pool = octx.enter_context(tc.tile_pool(name="h2fm", bufs=8))
        with ExitStack() as p5:
            x2p = p5.enter_context(tc.tile_pool(name="x2", bufs=4))
            ld = p5.enter_context(tc.tile_pool(name="ld5", bufs=3))
            x2_list = []
            for i in range(n_tok):
                r = slice(i * 128, (i + 1) * 128)
                xt = ld.tile([128, DM], FP32, tag="xr")
                nc.sync.dma_start(out=xt[:], in_=x_d[r, :])
                mt = ld.tile([128, DM], BF16, tag="mr")
                nc.sync.dma_start(out=mt[:], in_=m_ar[r, :])
                mtf = ld.tile([128, DM], FP32, tag="mrf")
                nc.any.tensor_copy(mtf[:], mt[:])
                x2 = x2p.tile([128, DM], FP32, tag="x2")
                nc.vector.tensor_add(x2[:], xt[:], mtf[:])
                nc.sync.dma_start(out=x2_out[r, :], in_=x2[:])
                x2_list.append(x2)
            h2fm = _layer_norm_stage(nc, tc, p5, x2_list, n_tok, ident_sb,
                                     ln2_g, ln2_b, h2fm_pool, L, "ln2")

        # ================= Phase 6: FF =================
        with ExitStack() as p6:
            mm6 = p6.enter_context(tc.tile_pool(name="mm6", bufs=4,
                                                space="PSUM"))
            wf = p6.enter_context(tc.tile_pool(name="wf", bufs=1))
            wff1_sb = []
            for k in range(8):
                t = wf.tile([128, 2 * FFS], BF16, tag=f"wff1{k}")
                nc.sync.dma_start(out=t[:],
                                  in_=w_ff1_d[k * 128:(k + 1) * 128, :])
                wff1_sb.append(t)
            wff2_sb = []
            for k in range(8):
                t = wf.tile([128, DM], BF16, tag=f"wff2{k}")
                nc.sync.dma_start(out=t[:],
                                  in_=w_ff2_d[k * 128:(k + 1) * 128, :])
                wff2_sb.append(t)
            bf1 = []
            for j in range(2 * FFS // 128):
                t = wf.tile([128, 1], FP32, tag=f"bf1_{j}")
                nc.sync.dma_start(out=t[:], in_=b_ff1_d[j * 128:(j + 1) * 128, :])
                bf1.append(t)

            agp = p6.enter_context(tc.tile_pool(name="ag", bufs=8))
            tmp6 = p6.enter_context(tc.tile_pool(name="tmp6", bufs=4))
            ag_sb = []
            for fa in range(8):
                agt = agp.tile([128, L], BF16, tag="ag")
                ag_sb.append(agt)
                for c in range(n_ch):
                    psA = mm.tile([128, CH], FP32, tag="mm")
                    for k in range(8):
                        nc.tensor.matmul(
                            psA[:], wff1_sb[k][:, fa * 128:(fa + 1) * 128],
                            h2fm[k][:, c * CH:(c + 1) * CH],
                            start=(k == 0), stop=(k == 7))
                    psG = mm6.tile([128, CH], FP32, tag="mm6")
                    for k in range(8):
                        nc.tensor.matmul(
                            psG[:],
                            wff1_sb[k][:, FFS + fa * 128:FFS + (fa + 1) * 128],
                            h2fm[k][:, c * CH:(c + 1) * CH],
                            start=(k == 0), stop=(k == 7))
                    aa = tmp6.tile([128, CH], BF16, tag="aa")
                    nc.scalar.activation(aa[:], psA[:], ACTF.Identity,
                                         bias=bf1[fa][:])
                    gg = tmp6.tile([128, CH], BF16, tag="gg")
                    _act_gelu(nc, tmp6, gg[:], psG[:],
                              bf1[FFS // 128 + fa][:])
                    nc.vector.tensor_mul(agt[:, c * CH:(c + 1) * CH],
                                         aa[:], gg[:])

            ffo = p6.enter_context(tc.tile_pool(name="ffo", bufs=2))
            for i in range(n_tok):
                fo = ffo.tile([128, DM], FP32, tag="fo")
                for nchk in range(2):
                    pool = mm if nchk == 0 else mm6
                    ps = pool.tile([128, 512], FP32, tag=pool.name)
                    for k in range(8):
                        nc.tensor.matmul(
                            ps[:], ag_sb[k][:, i * 128:(i + 1) * 128],
                            wff2_sb[k][:, nchk * 512:(nchk + 1) * 512],
                            start=(k == 0), stop=(k == 7))
                    nc.any.tensor_copy(fo[:, nchk * 512:(nchk + 1) * 512],
                                       ps[:])
                nc.sync.dma_start(out=ffp_out[i * 128:(i + 1) * 128, :],
                                  in_=fo[:])
    nc.compile()
    return nc


_NC_CACHE = {}


def _get_nc(L=L_FULL):
    if L not in _NC_CACHE:
        _NC_CACHE[L] = build_nc(L)
    return _NC_CACHE[L]


def make_in_maps(x, ln1_g, ln1_b, w_in, conv_w, conv_b, w_x, w_dt, b_dt,
                 a_log, d_skip, w_out, ln2_g, ln2_b, w_ff1, b_ff1, w_ff2,
                 b_ff2):
    x = np.asarray(x, np.float32)
    f32 = lambda a: np.ascontiguousarray(np.asarray(a, np.float32))
    bf = lambda a: np.ascontiguousarray(np.asarray(a, np.float32)).astype(NPBF16)
    ident = np.eye(128, dtype=np.float32).astype(NPBF16)
    a_neg = -np.exp(np.asarray(a_log, np.float32))
    in_maps = []
    for c in range(8):
        b, s = c // 4, c % 4
        ds = slice(s * DIS, (s + 1) * DIS)
        fs = slice(s * FFS, (s + 1) * FFS)
        gs = slice(FFI + s * FFS, FFI + (s + 1) * FFS)
        in_maps.append(dict(
            x=f32(x[b]),
            ln1_g=f32(ln1_g).reshape(DM, 1), ln1_b=f32(ln1_b).reshape(DM, 1),
            ln2_g=f32(ln2_g).reshape(DM, 1), ln2_b=f32(ln2_b).reshape(DM, 1),
            w_in=bf(np.concatenate(
                [w_in[:, s * DIS:(s + 1) * DIS],
                 w_in[:, DI + s * DIS:DI + (s + 1) * DIS]], axis=1)),
            conv_w=f32(conv_w[ds]), conv_b=f32(conv_b[ds]).reshape(DIS, 1),
            a_neg=f32(a_neg[ds]),
            w_x=bf(w_x[ds]), w_dt=bf(w_dt[:, ds]),
            b_dt=f32(b_dt[ds]).reshape(DIS, 1),
            d_skip=f32(d_skip[ds]).reshape(DIS, 1),
            w_out=bf(w_out[ds]),
            w_ff1=bf(np.concatenate([w_ff1[:, fs], w_ff1[:, gs]], axis=1)),
            b_ff1=f32(np.concatenate(
                [b_ff1[s * FFS:(s + 1) * FFS],
                 b_ff1[FFI + s * FFS:FFI + (s + 1) * FFS]])).reshape(2 * FFS, 1),
            w_ff2=bf(w_ff2[fs]),
            ident=ident,
        ))
    return in_maps


def combine_outputs(results, b_ff2, L=L_FULL):
    out = np.zeros((B_FULL, L, DM), np.float32)
    bff2 = np.asarray(b_ff2, np.float32)
    for b in range(B_FULL):
        acc = results[4 * b]["x2_out"].astype(np.float32).copy()
        for s in range(4):
            acc += results[4 * b + s]["ffp_out"].astype(np.float32)
        out[b] = acc + bff2[None, :]
    return out


def kernel(**inputs):
    nc = _get_nc(L_FULL)
    in_maps = make_in_maps(
        inputs["x"], inputs["ln1_g"], inputs["ln1_b"], inputs["w_in"],
        inputs["conv_w"], inputs["conv_b"], inputs["w_x"], inputs["w_dt"],
        inputs["b_dt"], inputs["a_log"], inputs["d_skip"], inputs["w_out"],
        inputs["ln2_g"], inputs["ln2_b"], inputs["w_ff1"], inputs["b_ff1"],
        inputs["w_ff2"], inputs["b_ff2"])
    res = run_bass_kernel_spmd(nc, in_maps, core_ids=list(range(8)))
    return combine_outputs(res.results, inputs["b_ff2"], L_FULL)
